# revision 1
# baseline (speedup 1.0000x reference)
"""Trainium2 Bass kernel for multi-head causal attention.

Problem: q, k, v of shape [4096, 16, 64] (seq, heads, head_dim) fp32.
  out = softmax(causal(q @ k^T / 8)) @ v, reshaped to [4096, 1024].

Sharding: heads are split across 8 NeuronCores (2 heads per core).
Each core runs the same SPMD Bass program on its own 2 heads; the host
concatenates the per-core [4096, 128] outputs along the feature dim.

Per-core algorithm (flash-attention style, S^T orientation):
  - Load Q, K as bf16 (SWDGE cast DMA) and transpose on the PE into
    qT/kT [128=(h,d), 4096] so head_dim sits on the partition axis.
  - Load V per head into vplus [128, 32*65] bf16: each 128-row k-block
    gets 64 V columns plus a ones column (fused softmax denominator).
  - For each 512-wide q group G, for each 128-wide k block j <= diag:
      mm1:  S^T[kj, qi] = kT_j^T.T @ qT_G  (both heads concurrently via
            PE row tiling: head0 rows 0-63, head1 rows 64-127)
      exp:  ScalarE activation Exp with scale=1/8, PSUM -> SBUF bf16,
            batched 3 k-blocks per instruction to amortize ACT overhead
      mask: diagonal blocks multiplied by precomputed 0/1 causal masks
      mm2:  O[qi, 64+1] += expS^T_chunk.T @ vplus_j  accumulated in PSUM
  - Normalize: reciprocal of the ones-column dot, row-scale, DMA out.

No distributed primitives are needed: sharding is purely host-side.
"""

import numpy as np

SEQ = 4096
NHEAD = 16
HDIM = 64
NCORES = 8
HPC = NHEAD // NCORES  # heads per core = 2
SCALE = 0.125

_NC_CACHE = {}
LAST_RESULT = {}


def build_attention_nc(seq=SEQ, hpc=HPC, hdim=HDIM):
    """Build the SPMD Bass program for one core handling `hpc` heads."""
    import concourse.bass as bass
    import concourse.mybir as mybir
    import concourse.tile as tile

    f32 = mybir.dt.float32
    bf16 = mybir.dt.bfloat16
    Exp = mybir.ActivationFunctionType.Exp

    assert hpc == 2 and hdim == 64, "layout hardcoded for 2 heads x 64 dim"
    assert seq % 512 == 0
    nt = seq // 128   # number of 128-row seq tiles
    ng = seq // 512   # number of 512-wide q groups

    nc = bass.Bass()
    q = nc.dram_tensor("q", [seq, hpc, hdim], f32, kind="ExternalInput").ap()
    k = nc.dram_tensor("k", [seq, hpc, hdim], f32, kind="ExternalInput").ap()
    v = nc.dram_tensor("v", [seq, hpc, hdim], f32, kind="ExternalInput").ap()
    o = nc.dram_tensor("o", [seq, hpc * hdim], f32, kind="ExternalOutput").ap()

    with tile.TileContext(nc) as tc:
        with (
            tc.tile_pool(name="persist", bufs=1) as persist,
            tc.tile_pool(name="pexp", bufs=3) as pexp_pool,
            tc.tile_pool(name="outp", bufs=8) as out_pool,
            tc.tile_pool(name="small", bufs=8) as small_pool,
        ):
            # ---- persistent SBUF tensors ----------------------------------
            # qT/kT: [(h,d)=128, seq] bf16 — contraction dim on partitions.
            qT = persist.tile([128, seq], bf16, tag="qT")
            kT = persist.tile([128, seq], bf16, tag="kT")
            # vplus per head: 32 blocks of [128, 65] = V block ++ ones col.
            vplus = [
                persist.tile([128, nt * (hdim + 1)], bf16, tag=f"vplus{h}", name=f"vplus{h}")
                for h in range(hpc)
            ]
            # Multiplicative 0/1 causal masks for the 4 diagonal
            # sub-positions t: mask_t[kj, qi] = 1 iff kj + 128*t <= qi.
            # Applied on the DVE to exp's output for diagonal blocks.
            masks = [persist.tile([128, 512], bf16, tag=f"mask{t}", name=f"mask{t}") for t in range(4)]

            def build_masks():
                for t in range(4):
                    nc.vector.memset(masks[t], 1.0)
                    # keep 1.0 where (-kj + qi - 128*t) >= 0, else fill 0.0
                    nc.gpsimd.affine_select(
                        out=masks[t][:],
                        in_=masks[t][:],
                        compare_op=mybir.AluOpType.is_ge,
                        fill=0.0,
                        base=-128 * t,
                        pattern=[[1, 512]],
                        channel_multiplier=-1,
                    )

            # ---- V load: cast fp32->bf16 during DMA, ones pre-memset ------
            def load_v():
                for h in range(hpc):
                    nc.vector.memset(vplus[h], 1.0)
                    nc.gpsimd.dma_start(
                        out=vplus[h].rearrange("p (t x) -> p t x", x=hdim + 1)[:, :, 0:hdim],
                        in_=v[:, h, :].rearrange("(t p) d -> p t d", p=128),
                    )

            # ---- Q/K load + transpose -------------------------------------
            # Cast-load [128 seq, 128 (h,d)] bf16 tiles (SWDGE cast DMA),
            # then transpose each on the PE (transpose-mode matmul) and copy
            # PSUM -> SBUF on the DVE. The transpose PSUM pool closes before
            # the main-loop PSUM pools open so the banks are reused.
            identity = persist.tile([128, 128], bf16, tag="identity")
            from concourse.masks import make_identity

            make_identity(nc, identity[:])
            chunk = min(8, nt)
            with (
                tc.tile_pool(name="ldstage", bufs=8) as ld_pool,
                tc.tile_pool(name="psum_tr", bufs=4, space="PSUM") as tr_pool,
            ):
                # interleave k/q chunks so G0's kT/qT tiles arrive early;
                # masks and V are queued behind the first chunk pair so the
                # gpsimd queue starts the critical staging DMAs immediately.
                for cstart in range(0, nt, chunk):
                    for src, dstT in ((k, kT), (q, qT)):
                        src_r = src.rearrange("(t p) h d -> p t (h d)", p=128)
                        st = ld_pool.tile([128, chunk * 128], bf16, tag="ldstage")
                        nc.gpsimd.dma_start(
                            out=st.rearrange("p (t x) -> p t x", x=128),
                            in_=src_r[:, cstart : cstart + chunk, :],
                        )
                        for tt in range(chunk):
                            tg = cstart + tt
                            ptr = tr_pool.tile([128, 128], bf16, tag="ptr", name="ptr")
                            nc.tensor.transpose(
                                ptr[:], st[:, tt * 128 : (tt + 1) * 128], identity[:]
                            )
                            nc.vector.tensor_copy(
                                dstT[:, tg * 128 : (tg + 1) * 128], ptr[:]
                            )
                    if cstart == 0:
                        build_masks()
                        load_v()

            # ---- main loop -------------------------------------------------
            with (
                tc.tile_pool(name="psum_s", bufs=2, space="PSUM") as psum_s_pool,
                tc.tile_pool(name="psum_o", bufs=1, space="PSUM") as psum_o_pool,
            ):
                _main_loop(
                    nc, mybir, ng, hdim, psum_s_pool, psum_o_pool, pexp_pool,
                    out_pool, small_pool, qT, kT, vplus, masks, identity, o,
                    hpc, Exp,
                )
    _split_multi_waits(nc)
    return nc


def _split_multi_waits(nc):
    """Walrus's codegen accepts at most one sync-wait per instruction on
    this toolchain. Hoist extra waits into standalone single-wait NoOps on
    the same engine queue (same semantics: the sequencer stalls in order)."""
    import concourse.mybir as mybir

    nsplit = 0
    for blk in nc.m.functions[0].blocks:
        newl = []
        for ins in blk.instructions:
            si = getattr(ins, "sync_info", None)
            if si is not None and si.on_wait and len(si.on_wait) > 1:
                waits = list(si.on_wait)
                for w in waits[:-1]:
                    newl.append(
                        mybir.InstNoOp(
                            name=f"{ins.name}-wsplit{nsplit}",
                            sync_info=mybir.SyncInfo(on_wait=[w], on_update=[]),
                            bass_nofuse=True,
                            engine=ins.engine,
                            ins=[],
                            outs=[],
                        )
                    )
                    nsplit += 1
                ins.sync_info = mybir.SyncInfo(
                    on_wait=[waits[-1]], on_update=list(si.on_update or [])
                )
            newl.append(ins)
        blk.instructions = newl
    return nsplit


def _main_loop(nc, mybir, ng, hdim, psum_s_pool, psum_o_pool, pexp_pool,
               out_pool, small_pool, qT, kT, vplus, masks, identity, o,
               hpc, Exp):
    SCALE = 0.125
    f32 = mybir.dt.float32
    bf16 = mybir.dt.bfloat16

    def emit_mm2s(st):
        """Deferred P@V accumulation for one jgroup (software pipelining:
        emitted after the NEXT jgroup's mm1/exp so the in-order PE queue
        always has independent work while ACT computes the current exp)."""
        G, jg, po, pes, njs, _ = st
        for h in range(hpc):
            pe = pes[h]
            for idx, j in enumerate(jg):
                t = j - 4 * G
                for c in range(4):
                    if t > c:
                        continue  # chunk fully masked -> zero
                    nc.tensor.matmul(
                        po[h][:, c * 128 : c * 128 + hdim + 1],
                        lhsT=pe[:, idx * 512 + c * 128 : idx * 512 + (c + 1) * 128],
                        rhs=vplus[h][:, j * 65 : j * 65 + hdim + 1],
                        start=(j == 0 and c == 0),
                        stop=(j == njs - 1 and c == 3),
                        skip_group_check=True,
                    )

    def emit_finals(G, po):
        for c in range(4):
            ob = out_pool.tile([128, hpc * hdim], f32, tag="ob", name="ob")
            for h in range(hpc):
                rec = small_pool.tile([128, 1], f32, tag="rec", name="rec")
                nc.vector.reciprocal(
                    rec, po[h][:, c * 128 + hdim : c * 128 + hdim + 1]
                )
                nc.vector.tensor_scalar_mul(
                    ob[:, h * hdim : (h + 1) * hdim],
                    po[h][:, c * 128 : c * 128 + hdim],
                    rec,
                )
            blk = G * 4 + c
            nc.sync.dma_start(
                out=o[blk * 128 : (blk + 1) * 128, :], in_=ob[:]
            )

    pending = None  # deferred mm2 state of the previous jgroup
    for G in range(ng):
        njs = 4 * G + 4  # causal: k blocks 0 .. 4G+3
        psum_o = [
            psum_o_pool.tile([128, 512], f32, tag=f"po{h}", name=f"po{h}")
            for h in range(hpc)
        ]
        jgroups = [list(range(s, min(s + 3, njs))) for s in range(0, njs, 3)]
        for gi, jg in enumerate(jgroups):
            w = len(jg)
            # mm1: S^T blocks, both heads interleaved for PE row
            # tiling concurrency (head0 rows 0-63, head1 rows 64-127).
            ps = [
                psum_s_pool.tile([128, 512 * w], f32, tag="ps", name="ps")
                for _ in range(hpc)
            ]
            for idx, j in enumerate(jg):
                t = j - 4 * G
                # Diagonal blocks: columns qi < 128*t are fully masked and
                # only ever read by mm2 chunks c < t, which are skipped, so
                # mm1 needn't compute them (saves streamed PE columns).
                # G0 keeps full width: its PSUM banks may hold stale
                # transpose-era bits and exp runs over the whole region.
                q0 = 128 * t if (t > 0 and G >= 1) else 0
                for h in range(hpc):
                    # explicit tile_position: head h occupies PE array rows
                    # h*64..h*64+63, so the two heads' K=64 matmuls execute
                    # concurrently on disjoint row groups.
                    nc.tensor.matmul(
                        ps[h][:, idx * 512 + q0 : (idx + 1) * 512],
                        lhsT=kT[h * 64 : (h + 1) * 64, j * 128 : (j + 1) * 128],
                        rhs=qT[h * 64 : (h + 1) * 64, G * 512 + q0 : (G + 1) * 512],
                        start=True,
                        stop=True,
                        tile_position=(h * 64, 0),
                    )
            pes = []
            for h in range(hpc):
                pe = pexp_pool.tile([128, 512 * w], bf16, tag="pexp", name="pexp")
                nc.scalar.activation(
                    out=pe[:], in_=ps[h][:], func=Exp, scale=SCALE
                )
                for idx, j in enumerate(jg):
                    t = j - 4 * G
                    if t >= 0:  # diagonal block: multiplicative causal mask
                        nc.vector.tensor_mul(
                            pe[:, idx * 512 : (idx + 1) * 512],
                            pe[:, idx * 512 : (idx + 1) * 512],
                            masks[t][:],
                        )
                pes.append(pe)
            if pending is not None:
                emit_mm2s(pending)
                if pending[5]:  # was the last jgroup of its G
                    emit_finals(pending[0], pending[2])
            pending = (G, jg, psum_o, pes, njs, gi == len(jgroups) - 1)
    if pending is not None:
        emit_mm2s(pending)
        emit_finals(pending[0], pending[2])


def _ensure_ntff_hook():
    """The image's antenv package lacks axon_hooks; provide it so
    run_bass_kernel_spmd's trace path works (or degrades gracefully)."""
    import sys
    import types

    try:
        import antenv.axon_hooks  # noqa: F401

        return
    except ImportError:
        pass
    mod = types.ModuleType("antenv.axon_hooks")
    state = {"hook": None}
    mod.set_axon_ntff_profile_hook = lambda h: state.__setitem__("hook", h)
    mod.get_axon_ntff_profile_hook = lambda: state["hook"]
    try:
        from trn_agent_boot.trn_boot import _ntff_profile_via_ctypes

        state["hook"] = _ntff_profile_via_ctypes("/opt/axon/libaxon_pjrt.so")
    except Exception:
        state["hook"] = None
    sys.modules["antenv.axon_hooks"] = mod


def kernel(q, k, v):
    """Full-input entry point: q, k, v [4096, 16, 64] fp32 -> [4096, 1024]."""
    import sys

    if "/opt/trn_rl_repo" not in sys.path:
        sys.path.insert(0, "/opt/trn_rl_repo")
    _ensure_ntff_hook()
    from concourse.bass_utils import run_bass_kernel_spmd

    q = np.asarray(q, dtype=np.float32)
    k = np.asarray(k, dtype=np.float32)
    v = np.asarray(v, dtype=np.float32)
    seq, nhead, hdim = q.shape

    if "nc" not in _NC_CACHE:
        _NC_CACHE["nc"] = build_attention_nc(seq=seq, hpc=HPC, hdim=hdim)
    nc = _NC_CACHE["nc"]

    in_maps = []
    for c in range(NCORES):
        hs = slice(c * HPC, (c + 1) * HPC)
        in_maps.append(
            {
                "q": np.ascontiguousarray(q[:, hs, :]),
                "k": np.ascontiguousarray(k[:, hs, :]),
                "v": np.ascontiguousarray(v[:, hs, :]),
            }
        )
    res = run_bass_kernel_spmd(nc, in_maps, core_ids=list(range(NCORES)))
    LAST_RESULT["exec_time_ns"] = res.exec_time_ns
    try:
        iat = res.instructions_and_trace
        LAST_RESULT["trace_path"] = iat[1] if iat else None
    except Exception:
        LAST_RESULT["trace_path"] = None
    outs = [res.results[c]["o"] for c in range(NCORES)]
    return np.concatenate(outs, axis=1)

